# revision 24
# baseline (speedup 1.0000x reference)
"""Trainium2 Bass kernel for LowRankSS2D.

Per core (8 cores, data-parallel over batch, 2 batches/core), ch-major
layouts [channels on partitions, L=4096 positions on free]:
  - in_proj/B_proj/gate/out_proj as chunked matmuls (fp32 bitcast float32r).
  - The 128-wide block scan collapses via the low-rank transition to a
    rank-64 recurrence q_t = q_{t-1}@S + c_{t-1} (S = U.T@V, c = xp@(B_w.T@V)),
    both directions x both batches fused into one 256-row chain, 63 steps.
  - Depthwise conv folded into the rank-64 expansion (shifted matmul taps,
    pairs packed to K=128) + 7 diag taps on bx; conv bias via evac bias.
  - LayerNorm: PE transposes to pos-major, bn_stats, per-partition scalar
    apply, transpose back with gamma/beta folded into the PSUM evacuation.
"""
import os
import numpy as np

os.environ.setdefault("MYCRO_LOCAL_CACHE", "1")

import concourse.bass as bass
import concourse.bacc as bacc
import concourse.mybir as mybir
import concourse.tile as tile
from concourse.bass_utils import run_bass_kernel_spmd

F32 = mybir.dt.float32
F32R = mybir.dt.float32r
F16 = mybir.dt.float16
AF = mybir.ActivationFunctionType
ALU = mybir.AluOpType

NCORES = 8
BPC = 2
L = 4096
PAD = 4
LP = L + 2 * PAD
CH = 256
ST = 128
RK = 64
NBLK = 64
T = 64
CHUNK = 512
NCH = L // CHUNK


def r32(ap):
    return ap.bitcast(F32R)


FULL = True


def build_kernel():
    nc = bacc.Bacc("TRN2", target_bir_lowering=False, debug=False)
    dp = lambda n, s, d=F32: nc.declare_dram_parameter(n, s, d, isOutput=False)

    x_d = dp("x16", [BPC, CH, L], F16)
    wtin_d = dp("wtin", [ST, 2, ST], F16)
    bin_d = dp("binp", [ST, 1])
    wtb_d = dp("wtb", [ST, ST], F16)
    bb_d = dp("bb", [ST, 1])
    bv_d = dp("bv", [ST, RK], F16)
    bvb_d = dp("bvb", [RK, 1])
    s64_d = dp("s64", [RK, RK])
    i64_d = dp("i64f16", [RK, RK], F16)
    id16_d = dp("id128f16", [ST, ST], F16)
    id32_d = dp("id128f32", [ST, ST])
    upk_d = dp("upk", [ST, 8, ST], F16)
    dcv_d = dp("dconv", [ST, 7, ST], F16)
    wout_d = dp("wout", [ST, CH], F16)
    bout_d = dp("bout", [ST, 2])
    cvb2_d = dp("cvb2", [ST, 1])
    gwt_d = dp("gwt16", [ST, 2, CH], F16)
    gb_d = dp("gb", [ST, 2])
    lng_d = dp("lng", [ST, 2])
    lnb_d = dp("lnb", [ST, 2])

    y_d = nc.declare_dram_parameter("y", [BPC, CH, L], F32, isOutput=True)
    st_d = nc.declare_dram_parameter("state", [BPC, L, ST], F32, isOutput=True)

    with tile.TileContext(nc) as tc:
        with (
            tc.tile_pool(name="const", bufs=1) as cp,
            tc.tile_pool(name="persist", bufs=1) as pp,
            tc.tile_pool(name="batch", bufs=1) as bp,
            tc.tile_pool(name="bx", bufs=2) as bxp,
            tc.tile_pool(name="small", bufs=3) as sp,
            tc.tile_pool(name="stg", bufs=2) as stg,
            tc.tile_pool(name="qsbp", bufs=3) as qp,
            tc.tile_pool(name="psum", bufs=2, space="PSUM") as ps,
        ):
            def cload(dram, shape, dt=F32, ap=None):
                t_ = cp.tile(shape, dt, tag="c_" + dram.name)
                nc.sync.dma_start(out=t_[:], in_=dram[:] if ap is None else ap)
                return t_

            wtin = cload(wtin_d, [ST, 2, ST], F16)
            bint = cload(bin_d, [ST, 1])
            wtb = cload(wtb_d, [ST, ST], F16)
            bbt = cload(bb_d, [ST, 1])
            bv = cload(bv_d, [ST, RK], F16)
            bvbt = cload(bvb_d, [RK, 1])
            s64 = cload(s64_d, [RK, RK])
            i64 = cload(i64_d, [RK, RK], F16)
            id16 = cload(id16_d, [ST, ST], F16)
            id32 = cload(id32_d, [ST, ST])
            upk = cload(upk_d, [ST, 8, ST], F16)
            dcv = cload(dcv_d, [ST, 7, ST], F16)
            wout = cload(wout_d, [ST, CH], F16)
            boutt = cload(bout_d, [ST, 2])
            cvb2 = cload(cvb2_d, [ST, 1])
            gwt = cload(gwt_d, [ST, 2, CH], F16)
            gbt = cload(gb_d, [ST, 2])
            lng = cload(lng_d, [ST, 2])
            lnb = cload(lnb_d, [ST, 2])
            s64r = cp.tile([RK, RK], F32R, tag="c_s64r")
            nc.vector.tensor_copy(s64r[:], s64[:])
            epst = cp.tile([ST, 1], F32)
            nc.vector.memset(epst[:], 1e-5)

            qpos = pp.tile([ST, 4 * LP], F16)
            c16 = pp.tile([RK, BPC * L], F16)
            nc.vector.memset(qpos[:], 0.0)

            xb16 = {}
            bx16 = {}
            w16 = {}
            xp32 = {}

            def ecopy(dst, src, bias=None, scale=None):
                kw = {}
                fn = AF.Copy
                if bias is not None:
                    kw["bias"] = bias
                    fn = AF.Identity
                if scale is not None:
                    kw["scale"] = scale
                nc.scalar.activation(dst, src, fn, **kw)

            for b in range(BPC):
                x16a = bxp.tile([ST, L], F16, tag="xb16a")
                x16b = bxp.tile([ST, L], F16, tag="xb16b")
                nc.sync.dma_start(out=x16a[:], in_=x_d[b, 0:ST, :])
                nc.sync.dma_start(out=x16b[:], in_=x_d[b, ST:CH, :])
                xb16[b] = (x16a, x16b)

                xp = bp.tile([ST, L], F16, tag="xp16")
                xp32[b] = xp
                for i in range(NCH):
                    sl = slice(i * CHUNK, (i + 1) * CHUNK)
                    pt = ps.tile([ST, CHUNK], F32, tag="ps")
                    nc.tensor.matmul(pt[:], wtin[:, 0, :], x16a[:, sl],
                                     start=True, stop=False)
                    nc.tensor.matmul(pt[:], wtin[:, 1, :], x16b[:, sl],
                                     start=False, stop=True)
                    ecopy(xp[:, sl], pt[:], bias=bint[:])

                # state_seq output: transpose xp to pos-major, stream out
                st_dst = st_d[b].rearrange("(t p) d -> p t d", p=ST)
                for g in range(8):
                    pt = ps.tile([ST, CHUNK], F32, tag="ps")
                    for j in range(4):
                        t_ = g * 4 + j
                        nc.tensor.matmul(pt[:, j * ST:(j + 1) * ST],
                                         xp[:, t_ * ST:(t_ + 1) * ST], id16[:],
                                         start=True, stop=True)
                    stc = stg.tile([ST, CHUNK], F32, tag="stch")
                    ecopy(stc[:], pt[:])
                    nc.sync.dma_start(
                        out=st_dst[:, g * 4:(g + 1) * 4, :],
                        in_=stc[:].rearrange("p (t d) -> p t d", t=4))

                bx = bxp.tile([ST, LP], F16, tag="bx16")
                bx16[b] = bx
                nc.vector.memset(bx[:, 0:PAD], 0.0)
                nc.vector.memset(bx[:, PAD + L:LP], 0.0)
                for i in range(NCH):
                    sl = slice(i * CHUNK, (i + 1) * CHUNK)
                    pt = ps.tile([ST, CHUNK], F32, tag="ps")
                    nc.tensor.matmul(pt[:], wtb[:], xp[:, sl],
                                     start=True, stop=True)
                    ecopy(bx[:, PAD + i * CHUNK:PAD + (i + 1) * CHUNK], pt[:],
                          bias=bbt[:])

                for i in range(NCH):
                    sl = slice(i * CHUNK, (i + 1) * CHUNK)
                    pt = ps.tile([RK, CHUNK], F32, tag="ps")
                    nc.tensor.matmul(pt[:], bv[:], xp[:, sl],
                                     start=True, stop=True)
                    ecopy(c16[:, b * L + i * CHUNK:b * L + (i + 1) * CHUNK],
                          pt[:], bias=bvbt[:])

            # ---- fused scan (63 steps) ----
            # psum/qsb col layout [64, 256]: (dir, b, blk); rl keeps orig blk order,
            # reading c at orig t = 64 - tau; block flip happens at scatter time.
            cpart = c16.ap[0][0]
            qpart = qpos.ap[0][0]

            def c_slice(d, tau):
                off = (tau - 1) if d == 0 else (64 - tau)
                return bass.AP(tensor=c16.tensor, offset=c16.offset + off,
                               ap=[[cpart, RK], [L, BPC], [NBLK, NBLK]])

            qprev = None
            for tau in range(1, T):
                pt = ps.tile([RK, 4 * NBLK], F32, tag="ps")
                nc.tensor.matmul(pt[:, 0:128], i64[:], c_slice(0, tau),
                                 start=True, stop=False)
                nc.tensor.matmul(pt[:, 128:256], i64[:], c_slice(1, tau),
                                 start=True, stop=(qprev is None))
                if qprev is not None:
                    nc.tensor.matmul(pt[:], s64r[:], qprev[:].bitcast(F32R),
                                     start=False, stop=True)
                qn = qp.tile([RK, 4 * NBLK], F32R, tag="qsb")
                ecopy(qn[:], pt[:])
                nc.vector.tensor_copy(
                    bass.AP(tensor=qpos.tensor, offset=qpos.offset + PAD + tau,
                            ap=[[qpart, RK], [LP, BPC], [NBLK, NBLK]]),
                    qn[:, 0:128])
                nc.vector.tensor_copy(
                    bass.AP(tensor=qpos.tensor,
                            offset=qpos.offset + 2 * LP + PAD + 63 * NBLK + (63 - tau),
                            ap=[[qpart, RK], [LP, BPC], [-NBLK, NBLK]]),
                    qn[:, 128:256])
                qprev = qn

            for db in range(4):
                nc.vector.tensor_copy(qpos[RK:ST, db * LP:db * LP + LP - 1],
                                      qpos[0:RK, db * LP + 1:db * LP + LP])

            # ---- combine / out_proj / gate / w ----
            OFFS = (2, 0, -2, -4)
            for b in range(BPC):
                bx = bx16[b]
                wa = bp.tile([ST, L], F16, tag="w16a")
                wb = bp.tile([ST, L], F16, tag="w16b")
                w16[b] = (wa, wb)
                for i in range(NCH):
                    c0 = i * CHUNK
                    zp = ps.tile([ST, CHUNK], F32, tag="zps")
                    for m in range(-3, 4):
                        nc.tensor.matmul(zp[:], dcv[:, m + 3, :],
                                         bx[:, PAD + c0 - m:PAD + c0 - m + CHUNK],
                                         start=(m == -3), stop=False)
                    for d in range(2):
                        base = (2 * d + b) * LP
                        for p in range(4):
                            o = base + PAD + c0 + OFFS[p]
                            nc.tensor.matmul(zp[:], upk[:, 4 * d + p, :],
                                             qpos[:, o:o + CHUNK],
                                             start=False, stop=(d == 1 and p == 3))
                    z32 = sp.tile([ST, CHUNK], F16, tag="z32")
                    ecopy(z32[:], zp[:], bias=cvb2[:])
                    for mt in range(2):
                        msl = slice(mt * ST, (mt + 1) * ST)
                        op_ = ps.tile([ST, CHUNK], F32, tag="ops")
                        nc.tensor.matmul(op_[:], wout[:, msl], z32[:],
                                         start=True, stop=True)
                        gp_ = ps.tile([ST, CHUNK], F32, tag="gps")
                        nc.tensor.matmul(gp_[:], gwt[:, 0, msl],
                                         xb16[b][0][:, c0:c0 + CHUNK],
                                         start=True, stop=False)
                        nc.tensor.matmul(gp_[:], gwt[:, 1, msl],
                                         xb16[b][1][:, c0:c0 + CHUNK],
                                         start=False, stop=True)
                        g16 = sp.tile([ST, CHUNK], F16, tag="g16")
                        nc.scalar.activation(g16[:], gp_[:], AF.Sigmoid,
                                             bias=gbt[:, mt:mt + 1])
                        t16 = sp.tile([ST, CHUNK], F16, tag="t16")
                        nc.vector.scalar_tensor_tensor(
                            t16[:], op_[:], boutt[:, mt:mt + 1], g16[:],
                            ALU.add, ALU.mult)
                        wdst = wa if mt == 0 else wb
                        nc.gpsimd.tensor_tensor(
                            wdst[:, c0:c0 + CHUNK], t16[:],
                            xb16[b][mt][:, c0:c0 + CHUNK], ALU.add)

            # ---- LayerNorm ----
            for b in range(BPC):
                wa, wb = w16[b]
                wpos = bp.tile([ST, 32 * CH], F16, tag="wpos")
                mva = bp.tile([ST, 64], F32, tag="mva")
                for t_ in range(32):
                    pt = ps.tile([ST, CH], F32, tag="ps")
                    nc.tensor.matmul(pt[:, 0:ST], wa[:, t_ * ST:(t_ + 1) * ST],
                                     id16[:], start=True, stop=True)
                    nc.tensor.matmul(pt[:, ST:CH], wb[:, t_ * ST:(t_ + 1) * ST],
                                     id16[:], start=True, stop=True)
                    wsl = slice(t_ * CH, (t_ + 1) * CH)
                    ecopy(wpos[:, wsl], pt[:])
                    st6 = sp.tile([ST, 6], F32, tag="st6")
                    nc.vector.bn_stats(out=st6[:], in_=wpos[:, wsl])
                    nc.vector.bn_aggr(out=mva[:, 2 * t_:2 * t_ + 2], in_=st6[:])
                rs = sp.tile([ST, 32], F32, tag="rs")
                nb = sp.tile([ST, 32], F32, tag="nb")
                sd = sp.tile([ST, 32], F32, tag="sd")
                mpart = mva.ap[0][0]
                mu_ap = bass.AP(tensor=mva.tensor, offset=mva.offset,
                                ap=[[mpart, ST], [2, 32]])
                var_ap = bass.AP(tensor=mva.tensor, offset=mva.offset + 1,
                                 ap=[[mpart, ST], [2, 32]])
                nc.scalar.activation(sd[:], var_ap, AF.Sqrt, bias=epst[:])
                nc.vector.reciprocal(rs[:], sd[:])
                nc.vector.tensor_tensor(nb[:], mu_ap, rs[:], ALU.mult)
                nc.vector.tensor_scalar_mul(nb[:], nb[:], -1.0)

                for g in range(8):
                    pa = ps.tile([ST, CHUNK], F32, tag="ps")
                    pb = ps.tile([ST, CHUNK], F32, tag="ps")
                    for j in range(4):
                        t_ = g * 4 + j
                        a16 = sp.tile([ST, CH], F16, tag="a16")
                        nc.scalar.activation(a16[:], wpos[:, t_ * CH:(t_ + 1) * CH],
                                             AF.Identity, bias=nb[:, t_:t_ + 1],
                                             scale=rs[:, t_:t_ + 1])
                        nc.tensor.matmul(pa[:, j * ST:(j + 1) * ST],
                                         a16[:, 0:ST], id16[:],
                                         start=True, stop=True)
                        nc.tensor.matmul(pb[:, j * ST:(j + 1) * ST],
                                         a16[:, ST:CH], id16[:],
                                         start=True, stop=True)
                    gsl = slice(g * CHUNK, (g + 1) * CHUNK)
                    ya = stg.tile([ST, CHUNK], F32, tag="ya")
                    yb = stg.tile([ST, CHUNK], F32, tag="yb")
                    nc.scalar.activation(ya[:], pa[:], AF.Identity,
                                         bias=lnb[:, 0:1], scale=lng[:, 0:1])
                    nc.scalar.activation(yb[:], pb[:], AF.Identity,
                                         bias=lnb[:, 1:2], scale=lng[:, 1:2])
                    nc.sync.dma_start(out=y_d[b, 0:ST, gsl], in_=ya[:])
                    nc.sync.dma_start(out=y_d[b, ST:CH, gsl], in_=yb[:])

    nc.finalize()
    return nc


_NC = None


def _get_nc():
    global _NC
    if _NC is None:
        _NC = build_kernel()
    return _NC


def _host_consts(in_proj_w, in_proj_b, U, V, B_w, B_b, out_w, out_b,
                 gate_w, gate_b, norm_g, norm_b, conv_w, conv_b):
    f32 = lambda a: np.ascontiguousarray(a, np.float32)
    f16 = lambda a: np.ascontiguousarray(a, np.float16)
    k = np.asarray(conv_w)[:, 0, :].astype(np.float64)
    kf = np.ascontiguousarray(k[:, ::-1])
    ksum = k + kf
    Uf = np.asarray(U, np.float64)
    Vf = np.asarray(V, np.float64)

    def umat(kd, m):
        if m == 0:
            coef = 1.0 + kd[:, 3]
        elif -3 <= m <= 3:
            coef = kd[:, 3 - m]
        else:
            return np.zeros((RK, ST))
        return (Uf * coef[:, None]).T

    upk = np.zeros((2, 4, ST, ST))
    for d, kd in ((0, k), (1, kf)):
        for pi, m in enumerate((-3, -1, 1, 3)):
            upk[d, pi, 0:RK, :] = umat(kd, m + 1)
            upk[d, pi, RK:ST, :] = umat(kd, m)

    dcv = np.zeros((7, ST, ST))
    for m in range(-3, 4):
        coef = ksum[:, 3 - m] if m != 0 else (2.0 + ksum[:, 3])
        dcv[m + 3] = np.diag(coef)

    return {
        "wtin": f16(np.asarray(in_proj_w, np.float64).T.reshape(2, ST, ST).transpose(1, 0, 2)),
        "binp": f32(np.asarray(in_proj_b)[:, None]),
        "wtb": f16(np.asarray(B_w).T),
        "bb": f32(np.asarray(B_b)[:, None]),
        "bv": f16(np.asarray(B_w, np.float64).T @ Vf),
        "bvb": f32((np.asarray(B_b, np.float64) @ Vf)[:, None]),
        "s64": f32(Uf.T @ Vf),
        "i64f16": f16(np.eye(RK)),
        "id128f16": f16(np.eye(ST)),
        "id128f32": f32(np.eye(ST)),
        "upk": f16(upk.transpose(2, 0, 1, 3).reshape(ST, 8, ST)),
        "dconv": f16(dcv.transpose(1, 0, 2)),
        "wout": f16(0.5 * np.asarray(out_w).T),
        "bout": f32(0.5 * np.asarray(out_b).reshape(2, ST).T),
        "cvb2": f32(2.0 * np.asarray(conv_b)[:, None]),
        "gwt16": f16(np.asarray(gate_w, np.float64).T.reshape(2, ST, CH).transpose(1, 0, 2)),
        "gb": f32(np.asarray(gate_b).reshape(2, ST).T),
        "lng": f32(np.asarray(norm_g).reshape(2, ST).T),
        "lnb": f32(np.asarray(norm_b).reshape(2, ST).T),
    }


def kernel(x, in_proj_w, in_proj_b, U, V, B_w, B_b, out_w, out_b,
           gate_w, gate_b, norm_g, norm_b, conv_w, conv_b, _bench=None):
    nc = _get_nc()
    consts = _host_consts(in_proj_w, in_proj_b, U, V, B_w, B_b, out_w, out_b,
                          gate_w, gate_b, norm_g, norm_b, conv_w, conv_b)
    xr = np.ascontiguousarray(np.asarray(x, np.float32).reshape(16, CH, L))
    in_maps = []
    for c in range(NCORES):
        m = dict(consts)
        m["x16"] = np.ascontiguousarray(xr[c * BPC:(c + 1) * BPC]).astype(np.float16)
        in_maps.append(m)
    kwargs = dict(_bench) if _bench else {}
    res = run_bass_kernel_spmd(nc, in_maps, list(range(NCORES)), **kwargs)
    y = np.concatenate([r["y"] for r in res.results], axis=0).reshape(16, CH, 64, 64)
    state = np.concatenate([r["state"] for r in res.results], axis=0)
    if _bench is not None:
        return (y, state), res
    return y, state
